# revision 34
# baseline (speedup 1.0000x reference)
"""Trainium2 Bass kernel: C = triu(A @ B), A/B upper-triangular 4096x4096 fp32.

Strategy (row-parallel over 8 cores, SPMD single program):
  * 32 row-blocks of 128 rows. Core c owns blocks {c, 8+c, 16+c, 24+c}
    ("slot" j = block 8j + c).
  * One uniform schedule for all cores: for column tile q (8 tiles of 512)
    and slot j, accumulate k-tiles k in [8j, 4q+3].  Per-core variation
    lives entirely in the DATA: the host packs A^T tiles per core and
    zero-fills tiles with k < own-block, so padded matmuls contribute
    exact zeros.  Since A and B are both upper-triangular, the lower
    triangle of C comes out exactly 0 - no masking needed.
  * A^T pack (80 tiles of 128x128) is cached in SBUF; B streams once per
    column tile with below-diagonal tiles skipped.
"""

import numpy as np
from contextlib import ExitStack

import concourse.mybir as mybir
import concourse.tile as tile
from concourse import bacc, bass_utils

N = 4096
P = 128
NCORES = 8
NSLOT = 4          # row-block slots per core
NQ = 8             # 512-wide output column tiles
QW = 512
NKT = 32           # 128-wide k tiles
KSTART = [0, 8, 16, 24]            # first k-tile per slot (min over cores)
ANT = [32, 24, 16, 8]              # k-tiles stored per slot
AOFF = [0, 32, 56, 72]             # slot offsets into the A pack
ATOT = 80                          # total packed A tiles per core

# (slot, qtile) pairs the program computes/writes, in emission order
PAIRS = [(j, q) for q in range(NQ) for j in range(NSLOT) if 4 * q + 4 > 8 * j]
NT = len(PAIRS)                    # 20 output tiles of 128x512 per core

# matmul dtype mode: "fp32r" (fast, ~11-bit mantissa), "bf16x3" (hi/lo
# 3-pass split, near-fp32 accuracy), "fp32" (exact, 4x slower PE)
MODE = "fp32r"

# pool buffer counts (double/triple buffering)
BUFS_B = 3
BUFS_O = 4
BUFS_PS = 8

_nc_cache = {}


def build_nc(mode=MODE, rep=1, variant="full"):
    """rep>1 repeats the whole compute (for dispatch-overhead-cancelling
    timing): T_hw ~= (T(rep=R) - T(rep=1)) / (R-1).
    variant: "full" | "nomm" (DMAs only) | "nodma" (matmuls only)."""
    if (mode, rep, variant) in _nc_cache:
        return _nc_cache[(mode, rep, variant)]
    two = 2 if mode == "bf16x3" else 1
    dt_in = {
        "fp32r": mybir.dt.float32r,
        "bf16x3": mybir.dt.bfloat16,
        "fp32": mybir.dt.float32,
    }[mode]

    nc = bacc.Bacc("TRN2", target_bir_lowering=False, debug=False,
                   num_devices=NCORES)
    # partition-major packed layouts (see pack_inputs): per-partition data is
    # contiguous so every DMA is 128 descriptors of large contiguous runs.
    # Apack row = h*P + p(k-within-tile), col = t*P + m  (40KB/partition)
    a_dram = nc.dram_tensor("Apack", [two * P, ATOT * P], dt_in,
                            kind="ExternalInput").ap()
    # B row = (h*NQ + q)*P + p, col = k*QW + n          (8KB runs/partition)
    b_dram = nc.dram_tensor("B", [two * NQ * P, NKT * QW], dt_in,
                            kind="ExternalInput").ap()
    c_dram = nc.dram_tensor("Cout", [NT * P, QW], mybir.dt.float32,
                            kind="ExternalOutput").ap()

    with tile.TileContext(nc) as tc:
        with ExitStack() as ctx:
            apool = ctx.enter_context(tc.tile_pool(name="apool", bufs=1))
            bpool = ctx.enter_context(tc.tile_pool(name="bpool", bufs=BUFS_B))
            opool = ctx.enter_context(tc.tile_pool(name="opool", bufs=BUFS_O))
            pspool = ctx.enter_context(
                tc.tile_pool(name="pspool", bufs=BUFS_PS, space="PSUM"))

            do_bdma = variant in ("full", "nomm", "vbdma")
            do_mm = variant in ("full", "nodma", "vmm")
            do_copy = variant in ("full", "nomm", "nodma", "vcopy")
            do_store = variant in ("full", "nomm", "nodma", "vstore")

            # A load split so early matmuls are gated only by the tiles they
            # read: slot0 k0..7 (feeds q=1/q=0) lands in ~1.5us, the rest
            # overlaps with the B stream.
            a_sb = apool.tile([P, two, ATOT, P], dt_in)
            for t0, t1 in [(0, 8), (8, 32), (32, ATOT)]:
                for h in range(two):
                    nc.sync.dma_start(
                        a_sb[:, h, t0:t1, :],
                        a_dram[h * P:(h + 1) * P, t0 * P:t1 * P].rearrange(
                            "p (t m) -> p t m", m=P))

            # micro variants: per rep emit n tiny ops, skip the main loop
            micro = variant.startswith("vd") or variant in ("vgps8", "vdve8")
            if micro:
                n_ops = (8 if variant in ("vgps8", "vdve8")
                         else int(variant[2:]))
                mpool = ctx.enter_context(tc.tile_pool(name="mp", bufs=16))
                for r in range(rep):
                    for i in range(n_ops):
                        mt = mpool.tile([P, QW], mybir.dt.float32, tag="mt",
                                        name=f"mt_{r}_{i}")
                        if variant == "vdve8":
                            src = a_sb[:, 0, 4 * i:4 * i + 4, :]
                            if dt_in == mybir.dt.float32r:
                                src = src.bitcast(mybir.dt.float32)
                            nc.vector.tensor_copy(
                                mt[:].rearrange("p (a b) -> p a b", a=4),
                                src)
                        elif variant == "vgps8":
                            nc.gpsimd.dma_start(
                                mt[:],
                                b_dram[i * P:(i + 1) * P, 0:QW]
                                .bitcast(mybir.dt.float32))
                        else:
                            nc.sync.dma_start(
                                mt[:],
                                b_dram[i * P:(i + 1) * P, 0:QW]
                                .bitcast(mybir.dt.float32))
            bt_fixed = None
            ot_fixed = None

            def _asrc_f32(j):
                src = a_sb[:, 0, 4 * j:4 * j + 4, :]
                if dt_in == mybir.dt.float32r:
                    src = src.bitcast(mybir.dt.float32)
                return src

            if variant == "vstore":
                ot_fixed = opool.tile([P, QW], mybir.dt.float32,
                                      name="ot_fixed")
                nc.vector.tensor_copy(
                    ot_fixed[:].rearrange("p (a b) -> p a b", a=4),
                    _asrc_f32(0))

            def _bsrc(h, kg, q):
                return b_dram[
                    (h * NQ + q) * P:(h * NQ + q + 1) * P,
                    4 * kg * QW:(4 * kg + 4) * QW,
                ].rearrange("p (ko n) -> p ko n", ko=4)

            def _load_diag_chunk(bt, q):
                # per k-row load only the valid columns [128i, 512) -
                # below-diagonal 128-blocks of B are zero
                for h in range(two):
                    for i in range(4):
                        row = (h * NQ + q) * P
                        col = (4 * q + i) * QW + 128 * i
                        nc.sync.dma_start(
                            bt[:, h, i, 128 * i:],
                            b_dram[row:row + P, col:col + QW - 128 * i])

            # q=0's only chunk (0.6MB) is consumed last (Q_ORDER ends on 0):
            # prefetch it into a dedicated buffer at the start so the tail
            # never waits on DMA
            # (tried: prefetching q=0's chunk at the head — model-worse by
            # 1.8us, the DMA stream is saturated so early bytes displace
            # the critical sequence)
            bt_q0 = None

            # q order: q=1 first (ready after the small A-head load), then
            # heaviest-to-lightest so the schedule drains into the tiny q=0
            # tail (4 matmuls + 1 copy + 1 store). Model-swept optimum.
            Q_ORDER = globals().get("_Q_ORDER_OVERRIDE") or \
                [1, 7, 6, 5, 4, 3, 2, 0]
            for _r, q in ([] if micro else
                          [(r, q) for r in range(rep) for q in Q_ORDER]):
                act = [j for j in range(NSLOT) if 4 * q + 4 > 8 * j]
                psums = {
                    j: pspool.tile([P, QW], mybir.dt.float32, tag="ps",
                                   name=f"ps_{_r}_{q}_{j}")
                    for j in act
                } if do_mm else {}
                kend = 4 * q + 3
                for kg in range(q + 1):
                    if do_mm and not do_bdma:
                        if bt_fixed is None:
                            bt_fixed = bpool.tile([P, two, 4, QW], dt_in,
                                                  tag="bt", name="bt_fixed")
                            for h in range(two):
                                nc.sync.dma_start(bt_fixed[:, h],
                                                  _bsrc(h, 0, 0))
                        bt = bt_fixed
                    elif do_bdma or variant == "vmin":
                        if variant == "vmin" and kg > 0:
                            continue
                        if bt_q0 is not None and q == 0:
                            bt = bt_q0
                        else:
                            bt = bpool.tile([P, two, 4, QW], dt_in,
                                            tag="bt")
                            if kg == q:
                                _load_diag_chunk(bt, q)
                            else:
                                for h in range(two):
                                    nc.sync.dma_start(bt[:, h],
                                                      _bsrc(h, kg, q))
                    else:
                        continue
                    if not do_mm:
                        continue
                    for i in range(4):
                        k = 4 * kg + i
                        # on the diagonal chunk only columns >= 128i are
                        # valid in SBUF (and B is zero left of them anyway)
                        c0 = 128 * i if kg == q else 0
                        for j in act:
                            if k < KSTART[j]:
                                continue
                            idx = AOFF[j] + (k - KSTART[j])
                            first = k == KSTART[j]
                            last = k == kend
                            if two == 1:
                                nc.tensor.matmul(
                                    psums[j][:, c0:], a_sb[:, 0, idx, :],
                                    bt[:, 0, i, c0:],
                                    start=first, stop=last)
                            else:
                                # hi@hi, hi@lo, lo@hi
                                for n3, (ha, hb) in enumerate(
                                        [(0, 0), (0, 1), (1, 0)]):
                                    nc.tensor.matmul(
                                        psums[j][:, c0:],
                                        a_sb[:, ha, idx, :],
                                        bt[:, hb, i, c0:],
                                        start=first and n3 == 0,
                                        stop=last and n3 == 2)
                for j in act:
                    if not (do_copy or do_store):
                        continue
                    t = PAIRS.index((j, q))
                    if variant == "vstore":
                        nc.sync.dma_start(
                            c_dram[t * P:(t + 1) * P, :], ot_fixed[:])
                        continue
                    ot = opool.tile([P, QW], mybir.dt.float32, tag="ot")
                    if do_mm:
                        nc.vector.tensor_copy(ot[:], psums[j][:])
                    else:
                        nc.vector.tensor_copy(
                            ot[:].rearrange("p (a b) -> p a b", a=4),
                            _asrc_f32(j))
                    if do_store:
                        # scalar (ACT) HWDGE ring: keeps compute-gated output
                        # stores out of the B-stream's SP FIFO
                        nc.scalar.dma_start(
                            c_dram[t * P:(t + 1) * P, :], ot[:])
    nc.compile()
    _nc_cache[(mode, rep, variant)] = nc
    return nc


def _split_bf16(x):
    import ml_dtypes
    hi = x.astype(ml_dtypes.bfloat16)
    lo = (x - hi.astype(np.float32)).astype(ml_dtypes.bfloat16)
    return hi, lo


def pack_inputs(A, B, mode=MODE):
    """Build per-core in_maps (partition-major packed layouts)."""
    A = np.ascontiguousarray(np.asarray(A, dtype=np.float32))
    B = np.ascontiguousarray(np.asarray(B, dtype=np.float32))
    two = 2 if mode == "bf16x3" else 1

    # B[128k+p, 512q+n] -> Bp[q, p, k, n] -> [NQ*P, NKT*QW]
    def _pack_b(x):
        return np.ascontiguousarray(
            x.reshape(NKT, P, NQ, QW).transpose(2, 1, 0, 3)
        ).reshape(NQ * P, NKT * QW)

    if mode == "bf16x3":
        hi, lo = _split_bf16(B)
        b_all = np.concatenate([_pack_b(hi), _pack_b(lo)], axis=0)
    else:
        b_all = _pack_b(B)

    in_maps = []
    for c in range(NCORES):
        ap = np.zeros((ATOT, P, P), np.float32)
        for j in range(NSLOT):
            b = 8 * j + c
            rb = P * b
            for k in range(max(KSTART[j], b), NKT):
                ap[AOFF[j] + k - KSTART[j]] = \
                    A[rb:rb + P, P * k:P * k + P].T
        # [t, p, m] -> [p, t, m] -> [P, ATOT*P]
        def _pack_a(x):
            return np.ascontiguousarray(
                x.transpose(1, 0, 2)).reshape(P, ATOT * P)

        if mode == "bf16x3":
            hi, lo = _split_bf16(ap)
            apk = np.concatenate([_pack_a(hi), _pack_a(lo)], axis=0)
        else:
            apk = _pack_a(ap)
        in_maps.append({"Apack": apk, "B": b_all})
    return in_maps


def unpack_output(results):
    C = np.zeros((N, N), np.float32)
    for c, r in enumerate(results):
        co = np.asarray(r["Cout"]).reshape(NT, P, QW)
        for t, (j, q) in enumerate(PAIRS):
            b = 8 * j + c
            C[P * b:P * b + P, QW * q:QW * q + QW] = co[t]
    return C


def kernel(A, B):
    nc = build_nc(MODE)
    in_maps = pack_inputs(A, B, MODE)
    res = bass_utils.run_bass_kernel_spmd(
        nc, in_maps, core_ids=list(range(NCORES)), trace=False)
    return unpack_output(res.results)
